# revision 49
# baseline (speedup 1.0000x reference)
"""Trainium2 Bass kernel for nn_Attention_53188874993896 (sparse_attention).

Math notes (derived from the reference):
  - pos_scores[b,h,s,t] = r[b,h,s] - r[b,h,t] + head_b[h] with
    r = p @ head_w[h].  The s-dependent part is constant along the softmax
    axis t, so pos_attn is independent of s: pos_attn[b,h,s,t] = w[b,h,t]
    where w = softmax_t(-r).  Its whole contribution to the output is a
    per-batch vector C[b,d] (rank-1 collapse).
  - blend a = (1-g)*attn + g*pos_attn already has rows summing to 1, so the
    reference's renormalization is an identity up to float rounding.
  - softmax without max-subtraction is safe: |scores| <~ 4.

Cost-model-driven structure (TimelineSim charges matmuls by OUTPUT FREE SIZE
only):
  - ctx is computed in [s, j] orientation with a fused denominator column
    (output free size 33/head) instead of [j, s] (free 257) -- ~8x fewer
    charged PE rows for the softmax reduction+apply stage.
  - blend rows are normalized via one stride-0-broadcast DVE multiply, then
    transposed back to [j, s] on the PE for the final projection.
  - x is transposed via the DMA xbar (dma_start_transpose), not the PE.
  - the pos branch contribution collapses to a per-batch row C[d] added to
    the output via a rank-1 ones matmul.

Sharding: data-parallel over batch B=64 across 8 cores (8 batches/core).
"""

import sys

sys.path.insert(0, "/opt/trn_rl_repo")

import numpy as np
import ml_dtypes

B, S, D, H, PD = 64, 256, 256, 8, 8
HD = D // H  # 32
P8 = D // 8  # 32
NCORES = 8
NB = B // NCORES  # batches per core
SCALE = 1.0 / np.sqrt(np.float32(HD))

bf16 = ml_dtypes.bfloat16

_CACHE = {}


def _build(nb, stage=99):
    import concourse.bass as bass
    import concourse.bacc as bacc
    import concourse.mybir as mybir
    from concourse.tile import TileContext

    fp32 = mybir.dt.float32
    bf = mybir.dt.bfloat16
    Exp = mybir.ActivationFunctionType.Exp

    nc = bacc.Bacc("TRN2", target_bir_lowering=False, debug=False)

    # ---- DRAM I/O ----
    x_d = nc.dram_tensor("x", [nb, S, D], fp32, kind="ExternalInput")
    pos_d = nc.dram_tensor("pos", [nb, S, PD], fp32, kind="ExternalInput")
    wq_d = nc.dram_tensor("wqT", [D, D], bf, kind="ExternalInput")  # [in,out]
    wk_d = nc.dram_tensor("wkT", [D, D], bf, kind="ExternalInput")
    vt_d = nc.dram_tensor("vT", [D, D], bf, kind="ExternalInput")  # (1-g) folded
    owt_d = nc.dram_tensor("owT", [D, D], bf, kind="ExternalInput")  # out_w.T
    owtg_d = nc.dram_tensor("owTg", [D, D], bf, kind="ExternalInput")  # * g/(1-g)
    pa_d = nc.dram_tensor("packA", [128, 512], bf, kind="ExternalInput")
    pb_d = nc.dram_tensor("packB", [32, 304], bf, kind="ExternalInput")
    pc_d = nc.dram_tensor("packC", [PD, 2], fp32, kind="ExternalInput")
    out_d = nc.dram_tensor("out", [nb, S, D], fp32, kind="ExternalOutput")

    with TileContext(nc) as tc:
        with (
            tc.tile_pool(name="wsb", bufs=1) as wsb,
            tc.tile_pool(name="xin", bufs=2) as xin,
            tc.tile_pool(name="xtp", bufs=2) as xtp,
            tc.tile_pool(name="qkv", bufs=4) as qkv,
            tc.tile_pool(name="esb", bufs=4) as esb,
            tc.tile_pool(name="bld", bufs=4) as bld,
            tc.tile_pool(name="small", bufs=4) as small,
            tc.tile_pool(name="osb", bufs=3) as osb,
            tc.tile_pool(name="ps", bufs=1, space="PSUM") as ps,
            tc.tile_pool(name="ps2", bufs=2, space="PSUM") as ps2,
        ):
            # ---- resident weights (SBUF); packed smalls load first so the
            # pos phase isn't starved behind the big projection weights ----
            pa_sb = wsb.tile([128, 512], bf, tag="packA")
            pb_sb = wsb.tile([32, 304], bf, tag="packB")
            pc_sb = wsb.tile([PD, 2], fp32, tag="packC")
            nc.sync.dma_start(out=pa_sb, in_=pa_d[:, :])
            nc.sync.dma_start(out=pb_sb, in_=pb_d[:, :])
            nc.sync.dma_start(out=pc_sb, in_=pc_d[:, :])
            id_sb = pa_sb[:, 0:128]
            ones_sb = pa_sb[:, 128:256]
            ind_sb = pa_sb[0:H, 256:512].rearrange("h (c p) -> h c p", c=2)
            w1_sb = pb_sb[0:PD, 0:PD]
            wn_sb = pb_sb[0:PD, PD:PD + H]
            outb_sb = pb_sb[0:1, 48:304]
            b1_sb = pc_sb[:, 0:1]
            hb2_sb = pc_sb[:, 1:2]
            wq_sb = wsb.tile([128, 2, D], bf, tag="wq")
            wk_sb = wsb.tile([128, 2, D], bf, tag="wk")
            vt_sb = wsb.tile([128, 2, D], bf, tag="vt")
            owt_sb = wsb.tile([128, 2, D], bf, tag="owt")
            owtg_sb = wsb.tile([128, 2, D], bf, tag="owtg")

            # ---- pos DMA first on the Pool queue, then x prefetches ----
            pos_all = wsb.tile([128, nb, 2, PD], bf, tag="posall")
            nc.gpsimd.dma_start(
                out=pos_all,
                in_=pos_d.rearrange("b (c p) i -> p b c i", p=128))

            x_tiles = {}
            xt_tiles = {}

            def fetch_x(b):
                x_bf = xin.tile([128, 2, D], bf, tag="x", name=f"x{b}")
                with tc.high_priority():
                    nc.gpsimd.dma_start(
                        out=x_bf, in_=x_d[b].rearrange("(c p) d -> p c d", p=128))
                # DMA xbar transpose: [s%128, d] -> [d%128, cs, cd, s']
                # (contiguous destination per call -- a strided dest breaks
                # the xbar path)
                xt_bf = xtp.tile([128, 2, 2, 128], bf, tag="xt", name=f"xt{b}")
                for cs in range(2):
                    nc.sync.dma_start_transpose(
                        out=xt_bf[:, cs, :, :], in_=x_bf[:, cs, :])
                x_tiles[b] = x_bf
                xt_tiles[b] = xt_bf

            for t, d in (
                (vt_sb, vt_d), (wq_sb, wq_d), (wk_sb, wk_d),
                (owt_sb, owt_d), (owtg_sb, owtg_d),
            ):
                nc.sync.dma_start(out=t, in_=d.rearrange("(c p) o -> p c o", p=128))
            if nb > 0:
                fetch_x(0)
            if nb > 1:
                fetch_x(1)

            projs = {}

            def proj(b):
                xt_bf = xt_tiles[b]
                # v projection: v[t, j] (rhs vT has (1-g) folded)
                v_ps = ps.tile([128, 2, D], fp32, tag="qkv", name=f"vp{b}")
                for ct in range(2):
                    for ci in range(2):
                        nc.tensor.matmul(
                            v_ps[:, ct, :],
                            lhsT=xt_bf[:, ct, ci, :],
                            rhs=vt_sb[:, ci, :],
                            start=(ci == 0), stop=(ci == 1))
                # v' with a ones column per head: [t%128, ct, h, 33]
                v_sb = qkv.tile([128, 2, H, HD + 1], bf, tag="v",
                                name=f"v{b}")
                nc.vector.tensor_copy(
                    v_sb[:, :, :, 0:HD],
                    v_ps.rearrange("p c (h e) -> p c h e", h=H))
                nc.gpsimd.memset(v_sb[:, :, :, HD:HD + 1], 1.0)
                # q/k projections -> [i%128, which, ci-chunk, s]
                qk_ps = ps.tile([128, 2, 2, S], fp32, tag="qkv",
                                name=f"qkp{b}")
                for wi, w_sb in ((0, wq_sb), (1, wk_sb)):
                    for cm in range(2):
                        for ci in range(2):
                            nc.tensor.matmul(
                                qk_ps[:, wi, cm, :],
                                lhsT=w_sb[:, ci, 128 * cm:128 * (cm + 1)],
                                rhs=xt_bf[:, :, ci, :],
                                start=(ci == 0), stop=(ci == 1))
                qkT_sb = qkv.tile([128, 2, 2, S], bf, tag="qk",
                                  name=f"qkT{b}")
                nc.vector.tensor_copy(qkT_sb, qk_ps)
                projs[b] = (v_sb, qkT_sb)

            # ---- pos branch: batched MLP, stage-major for pipelining ----
            # p = w2@h1 and r = hw^T@p fold into one matmul via WN = w2T@hwN.
            w_all = wsb.tile([H, nb, S], bf, tag="wall")  # exp(-r), unnorm
            wcol_sb = wsb.tile([128, nb, 2, H], bf, tag="wcol")
            wrecip_f = wsb.tile([H, nb], fp32, tag="wrecipf")
            wrecip_sb = wsb.tile([H, nb], bf, tag="wrecip")
            pairs = list(range(0, nb, 2))
            pt_l, posT_l, h1p_l, h1_l, rp_l = {}, {}, {}, {}, {}
            for b0 in pairs:
                w = min(2, nb - b0)
                pt_ps = ps2.tile([PD, 4, 128], bf, tag="scd",
                                 name=f"pt{b0}")
                for k in range(w):
                    for c in range(2):
                        nc.tensor.transpose(
                            pt_ps[:, 2 * k + c, :],
                            pos_all[:, b0 + k, c, :], id_sb)
                pt_l[b0] = (pt_ps, w)
            emitted_proj0 = []

            def _emit_proj0():
                if not emitted_proj0 and nb > 0:
                    emitted_proj0.append(1)
                    proj(0)

            for b0 in pairs:
                pt_ps, w = pt_l[b0]
                posT = small.tile([PD, 512], bf, tag="posT", bufs=4,
                                  name=f"posT{b0}")
                nc.vector.tensor_copy(
                    posT[:, 0:256 * w],
                    pt_ps[:, 0:2 * w, :].rearrange("i k t -> i (k t)"))
                posT_l[b0] = posT
            _emit_proj0()
            for b0 in pairs:
                w = min(2, nb - b0)
                h1_ps = ps2.tile([PD, 512], fp32, tag="scd", name=f"h1p{b0}")
                nc.tensor.matmul(
                    h1_ps[:, 0:256 * w], lhsT=w1_sb,
                    rhs=posT_l[b0][:, 0:256 * w], start=True, stop=True)
                h1p_l[b0] = h1_ps
            for b0 in pairs:
                w = min(2, nb - b0)
                h1 = small.tile([PD, 512], bf, tag="h1", bufs=4,
                                name=f"h1{b0}")
                nc.vector.tensor_scalar(
                    out=h1[:, 0:256 * w], in0=h1p_l[b0][:, 0:256 * w],
                    scalar1=b1_sb, scalar2=0.0,
                    op0=mybir.AluOpType.add, op1=mybir.AluOpType.max)
                h1_l[b0] = h1
            for b0 in pairs:
                w = min(2, nb - b0)
                r_ps = ps2.tile([H, 512], fp32, tag="scd", name=f"rp{b0}")
                nc.tensor.matmul(
                    r_ps[:, 0:256 * w], lhsT=wn_sb,
                    rhs=h1_l[b0][:, 0:256 * w], start=True, stop=True)
                rp_l[b0] = r_ps
            for b0 in pairs:
                w = min(2, nb - b0)
                nc.scalar.activation(
                    w_all[:, b0:b0 + w, :].rearrange("h b s -> h (b s)"),
                    rp_l[b0][:, 0:256 * w], Exp, bias=hb2_sb)
            # per-(b,h) normalizer computed per pair so batch 0 does not
            # wait for the whole pos phase
            ws_ps = ps.tile([H, nb], fp32, tag="aux")
            for b0 in pairs:
                w = min(2, nb - b0)
                wt_ps = ps2.tile([128, 4, H], bf, tag="scd", name=f"wt{b0}")
                for k in range(w):
                    for c in range(2):
                        nc.tensor.transpose(
                            wt_ps[:, 2 * k + c, :],
                            w_all[:, b0 + k, 128 * c:128 * (c + 1)],
                            id_sb[0:H, 0:H])
                nc.vector.tensor_copy(
                    wcol_sb[:, b0:b0 + w, :, :].rearrange(
                        "p b c h -> p (b c h)"),
                    wt_ps[:, 0:2 * w, :].rearrange("p k h -> p (k h)"))
                for k in range(w):
                    for ct in range(2):
                        nc.tensor.matmul(
                            ws_ps[:, b0 + k:b0 + k + 1],
                            lhsT=wcol_sb[:, b0 + k, ct, :],
                            rhs=ones_sb[:, 0:1],
                            start=(ct == 0), stop=(ct == 1))
                nc.vector.reciprocal_approx_fast(
                    wrecip_f[:, b0:b0 + w], ws_ps[:, b0:b0 + w])
                nc.vector.tensor_copy(
                    wrecip_sb[:, b0:b0 + w], wrecip_f[:, b0:b0 + w])

            # ---- main loop: head of batch b + split tail of batch b-1 ----
            tail = {}
            tail_bt = {}

            def emit_tail_bt(bp):
                (blend_sb, C_sb) = tail[bp]
                # blend^T via PE transposes -> [j%128, sc, cj, s']
                bt_ps = ps.tile([128, 2, 2, 128], bf, tag="aux")
                for sc in range(2):
                    for cj in range(2):
                        nc.tensor.transpose(
                            bt_ps[:, sc, cj, :],
                            blend_sb[:, sc, 4 * cj:4 * (cj + 1), :], id_sb)
                bt_sb = bld.tile([128, 2, 2, 128], bf, tag="bt")
                nc.vector.tensor_copy(bt_sb, bt_ps)
                tail_bt[bp] = bt_sb

            def emit_tail_f(bp):
                (blend_sb, C_sb) = tail.pop(bp)
                bt_sb = tail_bt.pop(bp)
                # final projection: C row + blend @ owT
                f_ps = ps.tile([128, 2, D], fp32, tag="f")
                for sc in range(2):
                    nc.tensor.matmul(f_ps[:, sc, :], lhsT=ones_sb[0:1, :],
                                     rhs=C_sb, start=True, stop=False)
                    for cj in range(2):
                        nc.tensor.matmul(
                            f_ps[:, sc, :], lhsT=bt_sb[:, sc, cj, :],
                            rhs=owt_sb[:, cj, :], start=False, stop=(cj == 1))
                o_sb = osb.tile([128, 2, D], fp32, tag="o")
                nc.vector.tensor_copy(o_sb, f_ps)
                nc.sync.dma_start(
                    out=out_d[bp].rearrange("(c p) d -> p c d", p=128), in_=o_sb)

            _emit_proj0()

            for b in range(nb):
                v_sb, qkT_sb = projs.pop(b)

                # ---- scores + exp, per (t-chunk, row-group pair) ----
                # HW constraint: every matmul writing into one PSUM bank must
                # use the same tile_position row; banks here hold (hg0, hg1)
                # slots of a single row group rg.
                e_tiles = [
                    esb.tile([128, 4, 2, S], bf, tag="e", name=f"e{b}_{ct}")
                    for ct in range(2)]  # [t', rg, hg, s]
                for rp in range(2):
                    for ct in range(2):
                        sc_ps = ps2.tile([128, 2, 2, S], fp32, tag="scd",
                                         name=f"s{b}_{ct}_{rp}")
                        for r2 in range(2):
                            rg = 2 * rp + r2
                            for hg in range(2):
                                nc.tensor.matmul(
                                    sc_ps[:, r2, hg, :],
                                    lhsT=qkT_sb[32 * rg:32 * (rg + 1), 1, hg,
                                                128 * ct:128 * (ct + 1)],
                                    rhs=qkT_sb[32 * rg:32 * (rg + 1), 0,
                                               hg, :],
                                    start=True, stop=True,
                                    tile_position=(32 * rg, 0))
                        nc.scalar.activation(
                            e_tiles[ct][:, 2 * rp:2 * (rp + 1), :, :], sc_ps,
                            Exp, scale=float(SCALE))

                # tail(b-1) part 1: fills ACT latency on PE
                if (b - 1) in tail:
                    emit_tail_bt(b - 1)

                # ---- pos-branch rank-1: vbar, wrecip replicate ----
                aux_ps = ps.tile([128, 260], fp32, tag="aux", name=f"aux{b}")
                for h in range(H):
                    cj, hh = h // 4, h % 4
                    for ct in range(2):
                        nc.tensor.matmul(
                            aux_ps[32 * hh:32 * (hh + 1), cj:cj + 1],
                            lhsT=v_sb[:, ct, h, 0:HD],
                            rhs=wcol_sb[:, b, ct, h:h + 1],
                            start=(ct == 0), stop=(ct == 1),
                            tile_position=(0, 32 * hh))
                for cj in range(2):
                    nc.tensor.matmul(
                        aux_ps[:, 2 + cj:3 + cj], lhsT=ind_sb[:, cj, :],
                        rhs=wrecip_sb[:, b:b + 1], start=True, stop=True)
                wr_sb = small.tile([128, 2], fp32, tag="wr")
                nc.vector.tensor_copy(wr_sb, aux_ps[:, 2:4])
                vbn_sb = small.tile([128, 2], bf, tag="vbn")
                nc.vector.tensor_mul(vbn_sb, aux_ps[:, 0:2], wr_sb)

                # ---- ctx + fused denominator: cd[s', sc, h, 33] ----
                cd_ps = ps2.tile([128, 2, H, 2 * HD], fp32, tag="scd",
                                 name=f"cd{b}")

                def cd_mm(heads):
                    for sc in range(2):
                        for h in heads:
                            for ct in range(2):
                                nc.tensor.matmul(
                                    cd_ps[:, sc, h, 0:HD + 1],
                                    lhsT=e_tiles[ct][:, h % 4, h // 4,
                                                     128 * sc:128 * (sc + 1)],
                                    rhs=v_sb[:, ct, h, :],
                                    start=(ct == 0), stop=(ct == 1))

                # tail(b-1) part 2 + next-batch projections fill the wait
                # for the last exps feeding cd rp1.
                if (b - 1) in tail:
                    emit_tail_f(b - 1)
                if b + 1 < nb:
                    proj(b + 1)

                cd_mm((0, 1, 4, 5))  # rg pair 0

                if b + 2 < nb:
                    fetch_x(b + 2)

                cd_mm((2, 3, 6, 7))  # rg pair 1

                # C row: pos contribution + bias, via vbn columns
                for cj in range(2):
                    nc.tensor.matmul(
                        aux_ps[0:1, 4:260], lhsT=vbn_sb[:, cj:cj + 1],
                        rhs=owtg_sb[:, cj, :], start=(cj == 0), stop=False)
                nc.tensor.matmul(
                    aux_ps[0:1, 4:260], lhsT=ones_sb[0:1, 0:1],
                    rhs=outb_sb, start=False, stop=True)
                C_sb = small.tile([1, D], bf, tag="C")
                nc.vector.tensor_copy(C_sb, aux_ps[0:1, 4:260])

                # ---- normalize: recip of den cols, stride-0 broadcast mul ----
                recip_sb = small.tile([128, 2, H, 1], fp32, tag="recip")
                nc.vector.reciprocal_approx_fast(
                    recip_sb.rearrange("p a h o -> p (a h) o"),
                    cd_ps[:, :, :, HD:HD + 1].rearrange("p a h o -> p (a h) o"))
                blend_sb = bld.tile([128, 2, H, HD], bf, tag="blend")
                r_bc = bass.AP(
                    tensor=recip_sb.tensor, offset=recip_sb.offset,
                    ap=list(recip_sb.ap[:3]) + [[0, HD]])
                nc.vector.tensor_mul(blend_sb, cd_ps[:, :, :, 0:HD], r_bc)

                tail[b] = (blend_sb, C_sb)

            if nb > 0:
                emit_tail_bt(nb - 1)
                emit_tail_f(nb - 1)

    nc.finalize()
    return nc


def _prep_inputs(inputs):
    g = 1.0 / (1.0 + np.exp(-inputs["gate"].astype(np.float64)))
    g = g.astype(np.float32)  # [H]
    omg_j = np.repeat(1.0 - g, HD)  # per j = 32h+d'
    gr_j = np.repeat(g / (1.0 - g), HD)

    wqT = inputs["Wq"].T.astype(bf16)
    wkT = inputs["Wk"].T.astype(bf16)
    vT = (inputs["v_embed"].reshape(D, D).T * omg_j[None, :]).astype(bf16)
    owT = inputs["out_w"].T.astype(bf16)
    owTg = (inputs["out_w"].T * gr_j[:, None]).astype(bf16)

    # packA [128, 512] bf16: identity | ones | head indicator
    packA = np.zeros((128, 512), dtype=np.float32)
    packA[:, 0:128] = np.eye(128, dtype=np.float32)
    packA[:, 128:256] = 1.0
    indH = np.zeros((H, 2, 128), dtype=np.float32)
    for h in range(H):
        indH[h, h // 4, 32 * (h % 4):32 * (h % 4 + 1)] = 1.0
    packA[0:H, 256:512] = indH.reshape(H, 256)
    packA = packA.astype(bf16)

    # packB [32, 304] bf16: w1T | w2T | hwNeg | out_b row
    packB = np.zeros((32, 304), dtype=np.float32)
    packB[0:PD, 0:PD] = inputs["pos_w1"].T
    # WN folds the second MLP layer and the per-head score weights:
    # r = hwN^T @ (w2 @ h1) = WN^T @ h1 with WN = w2T @ hwN
    packB[0:PD, PD:PD + H] = inputs["pos_w2"].T @ (-inputs["head_w"].T)
    packB[0:1, 48:304] = inputs["out_b"].reshape(1, D)
    packB = packB.astype(bf16)

    # packC [8, 2] fp32: pos_b1 col | -(head_w @ pos_b2) col
    packC = np.stack([
        inputs["pos_b1"].astype(np.float32),
        (-(inputs["head_w"] @ inputs["pos_b2"])).astype(np.float32),
    ], axis=1).astype(np.float32)

    shared = dict(wqT=wqT, wkT=wkT, vT=vT, owT=owT, owTg=owTg,
                  packA=packA, packB=packB, packC=packC)

    x = np.ascontiguousarray(inputs["x"], dtype=np.float32)
    pos = np.ascontiguousarray(inputs["pos"], dtype=np.float32)
    in_maps = []
    for c in range(NCORES):
        m = dict(shared)
        m["x"] = np.ascontiguousarray(x[c * NB:(c + 1) * NB])
        m["pos"] = np.ascontiguousarray(pos[c * NB:(c + 1) * NB])
        in_maps.append(m)
    return in_maps


def kernel(**inputs):
    from concourse.bass_utils import run_bass_kernel_spmd

    inputs = {k: np.asarray(v) for k, v in inputs.items()}
    if "nc" not in _CACHE:
        _CACHE["nc"] = _build(NB)
    in_maps = _prep_inputs(inputs)
    res = run_bass_kernel_spmd(_CACHE["nc"], in_maps, core_ids=list(range(NCORES)))
    out = np.concatenate([r["out"] for r in res.results], axis=0)
    return out.astype(np.float32)


# revision 50
# speedup vs baseline: 1.0523x; 1.0523x over previous
"""Trainium2 Bass kernel for nn_Attention_53188874993896 (sparse_attention).

Math notes (derived from the reference):
  - pos_scores[b,h,s,t] = r[b,h,s] - r[b,h,t] + head_b[h] with
    r = p @ head_w[h].  The s-dependent part is constant along the softmax
    axis t, so pos_attn is independent of s: pos_attn[b,h,s,t] = w[b,h,t]
    where w = softmax_t(-r).  Its whole contribution to the output is a
    per-batch vector C[b,d] (rank-1 collapse).
  - blend a = (1-g)*attn + g*pos_attn already has rows summing to 1, so the
    reference's renormalization is an identity up to float rounding.
  - softmax without max-subtraction is safe: |scores| <~ 4.

Cost-model-driven structure (TimelineSim charges matmuls by OUTPUT FREE SIZE
only):
  - ctx is computed in [s, j] orientation with a fused denominator column
    (output free size 33/head) instead of [j, s] (free 257) -- ~8x fewer
    charged PE rows for the softmax reduction+apply stage.
  - blend rows are normalized via one stride-0-broadcast DVE multiply, then
    transposed back to [j, s] on the PE for the final projection.
  - x is transposed via the DMA xbar (dma_start_transpose), not the PE.
  - the pos branch contribution collapses to a per-batch row C[d] added to
    the output via a rank-1 ones matmul.

Sharding: data-parallel over batch B=64 across 8 cores (8 batches/core).
"""

import sys

sys.path.insert(0, "/opt/trn_rl_repo")

import numpy as np
import ml_dtypes

B, S, D, H, PD = 64, 256, 256, 8, 8
HD = D // H  # 32
P8 = D // 8  # 32
NCORES = 8
NB = B // NCORES  # batches per core
SCALE = 1.0 / np.sqrt(np.float32(HD))

bf16 = ml_dtypes.bfloat16

_CACHE = {}


def _build(nb, stage=99):
    import concourse.bass as bass
    import concourse.bacc as bacc
    import concourse.mybir as mybir
    from concourse.tile import TileContext

    fp32 = mybir.dt.float32
    bf = mybir.dt.bfloat16
    Exp = mybir.ActivationFunctionType.Exp

    nc = bacc.Bacc("TRN2", target_bir_lowering=False, debug=False)

    # ---- DRAM I/O ----
    x_d = nc.dram_tensor("x", [nb, S, D], fp32, kind="ExternalInput")
    pos_d = nc.dram_tensor("pos", [nb, S, PD], fp32, kind="ExternalInput")
    wq_d = nc.dram_tensor("wqT", [D, D], bf, kind="ExternalInput")  # [in,out]
    wk_d = nc.dram_tensor("wkT", [D, D], bf, kind="ExternalInput")
    vt_d = nc.dram_tensor("vT", [D, D], bf, kind="ExternalInput")  # (1-g) folded
    owt_d = nc.dram_tensor("owT", [D, D], bf, kind="ExternalInput")  # out_w.T
    owtg_d = nc.dram_tensor("owTg", [D, D], bf, kind="ExternalInput")  # * g/(1-g)
    pa_d = nc.dram_tensor("packA", [128, 512], bf, kind="ExternalInput")
    pb_d = nc.dram_tensor("packB", [32, 304], bf, kind="ExternalInput")
    pc_d = nc.dram_tensor("packC", [PD, 2], fp32, kind="ExternalInput")
    out_d = nc.dram_tensor("out", [nb, S, D], fp32, kind="ExternalOutput")

    with TileContext(nc) as tc:
        with (
            tc.tile_pool(name="wsb", bufs=1) as wsb,
            tc.tile_pool(name="xin", bufs=2) as xin,
            tc.tile_pool(name="xtp", bufs=2) as xtp,
            tc.tile_pool(name="qkv", bufs=4) as qkv,
            tc.tile_pool(name="esb", bufs=4) as esb,
            tc.tile_pool(name="bld", bufs=4) as bld,
            tc.tile_pool(name="small", bufs=4) as small,
            tc.tile_pool(name="osb", bufs=3) as osb,
            tc.tile_pool(name="ps", bufs=1, space="PSUM") as ps,
            tc.tile_pool(name="ps2", bufs=2, space="PSUM") as ps2,
        ):
            # ---- resident weights (SBUF); packed smalls load first so the
            # pos phase isn't starved behind the big projection weights ----
            pa_sb = wsb.tile([128, 512], bf, tag="packA")
            pb_sb = wsb.tile([32, 304], bf, tag="packB")
            pc_sb = wsb.tile([PD, 2], fp32, tag="packC")
            nc.sync.dma_start(out=pa_sb, in_=pa_d[:, :])
            nc.sync.dma_start(out=pb_sb, in_=pb_d[:, :])
            nc.sync.dma_start(out=pc_sb, in_=pc_d[:, :])
            id_sb = pa_sb[:, 0:128]
            ones_sb = pa_sb[:, 128:256]
            ind_sb = pa_sb[0:H, 256:512].rearrange("h (c p) -> h c p", c=2)
            w1_sb = pb_sb[0:PD, 0:PD]
            wn_sb = pb_sb[0:PD, PD:PD + H]
            outb_sb = pb_sb[0:1, 48:304]
            b1_sb = pc_sb[:, 0:1]
            hb2_sb = pc_sb[:, 1:2]
            wq_sb = wsb.tile([128, 2, D], bf, tag="wq")
            wk_sb = wsb.tile([128, 2, D], bf, tag="wk")
            vt_sb = wsb.tile([128, 2, D], bf, tag="vt")
            owt_sb = wsb.tile([128, 2, D], bf, tag="owt")
            owtg_sb = wsb.tile([128, 2, D], bf, tag="owtg")

            # ---- pos DMA first on the Pool queue, then x prefetches ----
            pos_all = wsb.tile([128, nb, 2, PD], bf, tag="posall")
            nc.gpsimd.dma_start(
                out=pos_all,
                in_=pos_d.rearrange("b (c p) i -> p b c i", p=128))

            x_tiles = {}
            xt_tiles = {}

            def fetch_x(b):
                x_bf = xin.tile([128, 2, D], bf, tag="x", name=f"x{b}")
                with tc.high_priority():
                    nc.gpsimd.dma_start(
                        out=x_bf, in_=x_d[b].rearrange("(c p) d -> p c d", p=128))
                # DMA xbar transpose: [s%128, d] -> [d%128, cs, cd, s']
                # (contiguous destination per call -- a strided dest breaks
                # the xbar path)
                xt_bf = xtp.tile([128, 2, 2, 128], bf, tag="xt", name=f"xt{b}")
                for cs in range(2):
                    nc.sync.dma_start_transpose(
                        out=xt_bf[:, cs, :, :], in_=x_bf[:, cs, :])
                x_tiles[b] = x_bf
                xt_tiles[b] = xt_bf

            for t, d in (
                (vt_sb, vt_d), (wq_sb, wq_d), (wk_sb, wk_d),
                (owt_sb, owt_d), (owtg_sb, owtg_d),
            ):
                nc.sync.dma_start(out=t, in_=d.rearrange("(c p) o -> p c o", p=128))
            if nb > 0:
                fetch_x(0)
            if nb > 1:
                fetch_x(1)

            projs = {}

            def proj(b):
                xt_bf = xt_tiles[b]
                # v projection: v[t, j] (rhs vT has (1-g) folded)
                v_ps = ps.tile([128, 2, D], fp32, tag="qkv", name=f"vp{b}")
                for ct in range(2):
                    for ci in range(2):
                        nc.tensor.matmul(
                            v_ps[:, ct, :],
                            lhsT=xt_bf[:, ct, ci, :],
                            rhs=vt_sb[:, ci, :],
                            start=(ci == 0), stop=(ci == 1))
                # v' with a ones column per head: [t%128, ct, h, 33]
                v_sb = qkv.tile([128, 2, H, HD + 1], bf, tag="v",
                                name=f"v{b}")
                nc.vector.tensor_copy(
                    v_sb[:, :, :, 0:HD],
                    v_ps.rearrange("p c (h e) -> p c h e", h=H))
                nc.gpsimd.memset(v_sb[:, :, :, HD:HD + 1], 1.0)
                # q/k projections -> [i%128, which, ci-chunk, s]
                qk_ps = ps.tile([128, 2, 2, S], fp32, tag="qkv",
                                name=f"qkp{b}")
                for wi, w_sb in ((0, wq_sb), (1, wk_sb)):
                    for cm in range(2):
                        for ci in range(2):
                            nc.tensor.matmul(
                                qk_ps[:, wi, cm, :],
                                lhsT=w_sb[:, ci, 128 * cm:128 * (cm + 1)],
                                rhs=xt_bf[:, :, ci, :],
                                start=(ci == 0), stop=(ci == 1))
                qkT_sb = qkv.tile([128, 2, 2, S], bf, tag="qk",
                                  name=f"qkT{b}")
                nc.vector.tensor_copy(qkT_sb, qk_ps)
                projs[b] = (v_sb, qkT_sb)

            # ---- pos branch: batched MLP, stage-major for pipelining ----
            # p = w2@h1 and r = hw^T@p fold into one matmul via WN = w2T@hwN.
            w_all = wsb.tile([H, nb, S], bf, tag="wall")  # exp(-r), unnorm
            wcol_sb = wsb.tile([128, nb, 2, H], bf, tag="wcol")
            wrecip_f = wsb.tile([H, nb], fp32, tag="wrecipf")
            wrecip_sb = wsb.tile([H, nb], bf, tag="wrecip")
            pairs = list(range(0, nb, 2))
            pt_l, posT_l, h1p_l, h1_l, rp_l = {}, {}, {}, {}, {}
            for b0 in pairs:
                w = min(2, nb - b0)
                pt_ps = ps2.tile([PD, 4, 128], bf, tag="scd",
                                 name=f"pt{b0}")
                for k in range(w):
                    for c in range(2):
                        nc.tensor.transpose(
                            pt_ps[:, 2 * k + c, :],
                            pos_all[:, b0 + k, c, :], id_sb)
                pt_l[b0] = (pt_ps, w)
            emitted_proj0 = []

            def _emit_proj0():
                if not emitted_proj0 and nb > 0:
                    emitted_proj0.append(1)
                    proj(0)

            for b0 in pairs:
                pt_ps, w = pt_l[b0]
                posT = small.tile([PD, 512], bf, tag="posT", bufs=4,
                                  name=f"posT{b0}")
                nc.vector.tensor_copy(
                    posT[:, 0:256 * w],
                    pt_ps[:, 0:2 * w, :].rearrange("i k t -> i (k t)"))
                posT_l[b0] = posT
            _emit_proj0()
            for b0 in pairs:
                w = min(2, nb - b0)
                h1_ps = ps2.tile([PD, 512], fp32, tag="scd", name=f"h1p{b0}")
                nc.tensor.matmul(
                    h1_ps[:, 0:256 * w], lhsT=w1_sb,
                    rhs=posT_l[b0][:, 0:256 * w], start=True, stop=True)
                h1p_l[b0] = h1_ps
            for b0 in pairs:
                w = min(2, nb - b0)
                h1 = small.tile([PD, 512], bf, tag="h1", bufs=4,
                                name=f"h1{b0}")
                nc.vector.tensor_scalar(
                    out=h1[:, 0:256 * w], in0=h1p_l[b0][:, 0:256 * w],
                    scalar1=b1_sb, scalar2=0.0,
                    op0=mybir.AluOpType.add, op1=mybir.AluOpType.max)
                h1_l[b0] = h1
            for b0 in pairs:
                w = min(2, nb - b0)
                r_ps = ps2.tile([H, 512], fp32, tag="scd", name=f"rp{b0}")
                nc.tensor.matmul(
                    r_ps[:, 0:256 * w], lhsT=wn_sb,
                    rhs=h1_l[b0][:, 0:256 * w], start=True, stop=True)
                rp_l[b0] = r_ps
            for b0 in pairs:
                w = min(2, nb - b0)
                nc.scalar.activation(
                    w_all[:, b0:b0 + w, :].rearrange("h b s -> h (b s)"),
                    rp_l[b0][:, 0:256 * w], Exp, bias=hb2_sb)
            for b0 in pairs:
                w = min(2, nb - b0)
                wt_ps = ps2.tile([128, 4, H], bf, tag="scd", name=f"wt{b0}")
                for k in range(w):
                    for c in range(2):
                        nc.tensor.transpose(
                            wt_ps[:, 2 * k + c, :],
                            w_all[:, b0 + k, 128 * c:128 * (c + 1)],
                            id_sb[0:H, 0:H])
                nc.vector.tensor_copy(
                    wcol_sb[:, b0:b0 + w, :, :].rearrange(
                        "p b c h -> p (b c h)"),
                    wt_ps[:, 0:2 * w, :].rearrange("p k h -> p (k h)"))
            # per-(b,h) normalizer of w: wsum = sum_t w -> reciprocal
            ws_ps = ps.tile([H, nb], fp32, tag="aux")
            for b in range(nb):
                for ct in range(2):
                    nc.tensor.matmul(
                        ws_ps[:, b:b + 1], lhsT=wcol_sb[:, b, ct, :],
                        rhs=ones_sb[:, 0:1],
                        start=(ct == 0), stop=(ct == 1))
            nc.vector.reciprocal_approx_fast(wrecip_f, ws_ps)
            nc.vector.tensor_copy(wrecip_sb, wrecip_f)

            # ---- main loop: head of batch b + split tail of batch b-1 ----
            tail = {}
            tail_bt = {}

            def emit_tail_bt(bp):
                (blend_sb, C_sb) = tail[bp]
                # blend^T via PE transposes -> [j%128, sc, cj, s']
                bt_ps = ps.tile([128, 2, 2, 128], bf, tag="aux")
                for sc in range(2):
                    for cj in range(2):
                        nc.tensor.transpose(
                            bt_ps[:, sc, cj, :],
                            blend_sb[:, sc, 4 * cj:4 * (cj + 1), :], id_sb)
                bt_sb = bld.tile([128, 2, 2, 128], bf, tag="bt")
                nc.vector.tensor_copy(bt_sb, bt_ps)
                tail_bt[bp] = bt_sb

            def emit_tail_f(bp):
                (blend_sb, C_sb) = tail.pop(bp)
                bt_sb = tail_bt.pop(bp)
                # final projection: C row + blend @ owT
                f_ps = ps.tile([128, 2, D], fp32, tag="f")
                for sc in range(2):
                    nc.tensor.matmul(f_ps[:, sc, :], lhsT=ones_sb[0:1, :],
                                     rhs=C_sb, start=True, stop=False)
                    for cj in range(2):
                        nc.tensor.matmul(
                            f_ps[:, sc, :], lhsT=bt_sb[:, sc, cj, :],
                            rhs=owt_sb[:, cj, :], start=False, stop=(cj == 1))
                o_sb = osb.tile([128, 2, D], fp32, tag="o")
                nc.vector.tensor_copy(o_sb, f_ps)
                nc.sync.dma_start(
                    out=out_d[bp].rearrange("(c p) d -> p c d", p=128), in_=o_sb)

            _emit_proj0()

            for b in range(nb):
                v_sb, qkT_sb = projs.pop(b)

                # ---- scores + exp, per (t-chunk, row-group pair) ----
                # HW constraint: every matmul writing into one PSUM bank must
                # use the same tile_position row; banks here hold (hg0, hg1)
                # slots of a single row group rg.
                e_tiles = [
                    esb.tile([128, 4, 2, S], bf, tag="e", name=f"e{b}_{ct}")
                    for ct in range(2)]  # [t', rg, hg, s]
                for rp in range(2):
                    for ct in range(2):
                        sc_ps = ps2.tile([128, 2, 2, S], fp32, tag="scd",
                                         name=f"s{b}_{ct}_{rp}")
                        for r2 in range(2):
                            rg = 2 * rp + r2
                            for hg in range(2):
                                nc.tensor.matmul(
                                    sc_ps[:, r2, hg, :],
                                    lhsT=qkT_sb[32 * rg:32 * (rg + 1), 1, hg,
                                                128 * ct:128 * (ct + 1)],
                                    rhs=qkT_sb[32 * rg:32 * (rg + 1), 0,
                                               hg, :],
                                    start=True, stop=True,
                                    tile_position=(32 * rg, 0))
                        nc.scalar.activation(
                            e_tiles[ct][:, 2 * rp:2 * (rp + 1), :, :], sc_ps,
                            Exp, scale=float(SCALE))

                # tail(b-1) part 1: fills ACT latency on PE
                if (b - 1) in tail:
                    emit_tail_bt(b - 1)

                # ---- pos-branch rank-1: vbar, wrecip replicate ----
                aux_ps = ps.tile([128, 260], fp32, tag="aux", name=f"aux{b}")
                for h in range(H):
                    cj, hh = h // 4, h % 4
                    for ct in range(2):
                        nc.tensor.matmul(
                            aux_ps[32 * hh:32 * (hh + 1), cj:cj + 1],
                            lhsT=v_sb[:, ct, h, 0:HD],
                            rhs=wcol_sb[:, b, ct, h:h + 1],
                            start=(ct == 0), stop=(ct == 1),
                            tile_position=(0, 32 * hh))
                for cj in range(2):
                    nc.tensor.matmul(
                        aux_ps[:, 2 + cj:3 + cj], lhsT=ind_sb[:, cj, :],
                        rhs=wrecip_sb[:, b:b + 1], start=True, stop=True)
                wr_sb = small.tile([128, 2], fp32, tag="wr")
                nc.vector.tensor_copy(wr_sb, aux_ps[:, 2:4])
                vbn_sb = small.tile([128, 2], bf, tag="vbn")
                nc.vector.tensor_mul(vbn_sb, aux_ps[:, 0:2], wr_sb)

                # ---- ctx + fused denominator: cd[s', sc, h, 33] ----
                cd_ps = ps2.tile([128, 2, H, 2 * HD], fp32, tag="scd",
                                 name=f"cd{b}")

                def cd_mm(heads):
                    for sc in range(2):
                        for h in heads:
                            for ct in range(2):
                                nc.tensor.matmul(
                                    cd_ps[:, sc, h, 0:HD + 1],
                                    lhsT=e_tiles[ct][:, h % 4, h // 4,
                                                     128 * sc:128 * (sc + 1)],
                                    rhs=v_sb[:, ct, h, :],
                                    start=(ct == 0), stop=(ct == 1))

                # tail(b-1) part 2 + next-batch projections fill the wait
                # for the last exps feeding cd rp1.
                if (b - 1) in tail:
                    emit_tail_f(b - 1)
                if b + 1 < nb:
                    proj(b + 1)

                cd_mm((0, 1, 4, 5))  # rg pair 0

                if b + 2 < nb:
                    fetch_x(b + 2)

                cd_mm((2, 3, 6, 7))  # rg pair 1

                # C row: pos contribution + bias, via vbn columns
                for cj in range(2):
                    nc.tensor.matmul(
                        aux_ps[0:1, 4:260], lhsT=vbn_sb[:, cj:cj + 1],
                        rhs=owtg_sb[:, cj, :], start=(cj == 0), stop=False)
                nc.tensor.matmul(
                    aux_ps[0:1, 4:260], lhsT=ones_sb[0:1, 0:1],
                    rhs=outb_sb, start=False, stop=True)
                C_sb = small.tile([1, D], bf, tag="C")
                nc.vector.tensor_copy(C_sb, aux_ps[0:1, 4:260])

                # ---- normalize: recip of den cols, stride-0 broadcast mul ----
                recip_sb = small.tile([128, 2, H, 1], fp32, tag="recip")
                nc.vector.reciprocal_approx_fast(
                    recip_sb.rearrange("p a h o -> p (a h) o"),
                    cd_ps[:, :, :, HD:HD + 1].rearrange("p a h o -> p (a h) o"))
                blend_sb = bld.tile([128, 2, H, HD], bf, tag="blend")
                r_bc = bass.AP(
                    tensor=recip_sb.tensor, offset=recip_sb.offset,
                    ap=list(recip_sb.ap[:3]) + [[0, HD]])
                nc.vector.tensor_mul(blend_sb, cd_ps[:, :, :, 0:HD], r_bc)

                tail[b] = (blend_sb, C_sb)

            if nb > 0:
                emit_tail_bt(nb - 1)
                emit_tail_f(nb - 1)

    nc.finalize()
    return nc


def _prep_inputs(inputs):
    g = 1.0 / (1.0 + np.exp(-inputs["gate"].astype(np.float64)))
    g = g.astype(np.float32)  # [H]
    omg_j = np.repeat(1.0 - g, HD)  # per j = 32h+d'
    gr_j = np.repeat(g / (1.0 - g), HD)

    wqT = inputs["Wq"].T.astype(bf16)
    wkT = inputs["Wk"].T.astype(bf16)
    vT = (inputs["v_embed"].reshape(D, D).T * omg_j[None, :]).astype(bf16)
    owT = inputs["out_w"].T.astype(bf16)
    owTg = (inputs["out_w"].T * gr_j[:, None]).astype(bf16)

    # packA [128, 512] bf16: identity | ones | head indicator
    packA = np.zeros((128, 512), dtype=np.float32)
    packA[:, 0:128] = np.eye(128, dtype=np.float32)
    packA[:, 128:256] = 1.0
    indH = np.zeros((H, 2, 128), dtype=np.float32)
    for h in range(H):
        indH[h, h // 4, 32 * (h % 4):32 * (h % 4 + 1)] = 1.0
    packA[0:H, 256:512] = indH.reshape(H, 256)
    packA = packA.astype(bf16)

    # packB [32, 304] bf16: w1T | w2T | hwNeg | out_b row
    packB = np.zeros((32, 304), dtype=np.float32)
    packB[0:PD, 0:PD] = inputs["pos_w1"].T
    # WN folds the second MLP layer and the per-head score weights:
    # r = hwN^T @ (w2 @ h1) = WN^T @ h1 with WN = w2T @ hwN
    packB[0:PD, PD:PD + H] = inputs["pos_w2"].T @ (-inputs["head_w"].T)
    packB[0:1, 48:304] = inputs["out_b"].reshape(1, D)
    packB = packB.astype(bf16)

    # packC [8, 2] fp32: pos_b1 col | -(head_w @ pos_b2) col
    packC = np.stack([
        inputs["pos_b1"].astype(np.float32),
        (-(inputs["head_w"] @ inputs["pos_b2"])).astype(np.float32),
    ], axis=1).astype(np.float32)

    shared = dict(wqT=wqT, wkT=wkT, vT=vT, owT=owT, owTg=owTg,
                  packA=packA, packB=packB, packC=packC)

    x = np.ascontiguousarray(inputs["x"], dtype=np.float32)
    pos = np.ascontiguousarray(inputs["pos"], dtype=np.float32)
    in_maps = []
    for c in range(NCORES):
        m = dict(shared)
        m["x"] = np.ascontiguousarray(x[c * NB:(c + 1) * NB])
        m["pos"] = np.ascontiguousarray(pos[c * NB:(c + 1) * NB])
        in_maps.append(m)
    return in_maps


def kernel(**inputs):
    from concourse.bass_utils import run_bass_kernel_spmd

    inputs = {k: np.asarray(v) for k, v in inputs.items()}
    if "nc" not in _CACHE:
        _CACHE["nc"] = _build(NB)
    in_maps = _prep_inputs(inputs)
    res = run_bass_kernel_spmd(_CACHE["nc"], in_maps, core_ids=list(range(NCORES)))
    out = np.concatenate([r["out"] for r in res.results], axis=0)
    return out.astype(np.float32)
